# revision 7
# baseline (speedup 1.0000x reference)
"""GAT 2-layer kernel for 8 TRN2 NeuronCores.

Strategy:
- dst-shard edges across 8 cores (6250 nodes/core); edge softmax + scatter local.
- GEMM0 replicated on every core (each core materializes full h0 table, rotated
  rank order so own shard comes first). GEMM1 sharded + 7 chunked AllGathers
  overlapped with the layer-0 edge phase.
- h table rows [528] bf16: cols 0..511 = h, 512..515 = el, rest pad.
- Edge phase per 128-node block: indirect-DMA gather h[src] rows, er via
  transposed-one-hot fp8 matmul + el added via identity matmul (PSUM), Prelu+Exp
  on ACT, alpha-weighted messages via per-head tensor_scalar, scatter-add +
  softmax denominator via one-hot fp8 matmuls accumulating in PSUM.
"""

import numpy as np
import ml_dtypes

N, E, IN_DIM, H, D = 50000, 600000, 256, 4, 128
NCORES = 8
SH = N // NCORES            # 6250 real rows per core
NB = 49                     # node blocks per core (49*128 = 6272)
RPC = NB * 128              # 6272 padded rows per core
NPAD = NCORES * RPC         # 50176
COLS = 528                  # 512 h + 4 el + 12 pad (1056B rows, 32B aligned)
CHB = 7                     # blocks per AllGather chunk (7 chunks x 7 blocks)
CHROWS = CHB * 128          # 896
PAD_EL = -1e4

bf16 = ml_dtypes.bfloat16
f8 = ml_dtypes.float8_e4m3

_cache = {}


def _host_prep(x, src, dst, W0, al0, ar0, W1, al1, ar1):
    f32 = np.float32
    x = np.asarray(x, f32)
    src = np.asarray(src, np.int32)
    dst = np.asarray(dst, np.int32)
    W0 = np.asarray(W0, f32)
    W1 = np.asarray(W1, f32)
    al0 = np.asarray(al0, f32)
    ar0 = np.asarray(ar0, f32)
    al1 = np.asarray(al1, f32)
    ar1 = np.asarray(ar1, f32)

    # Extended weights: cols 512..515 el-projection, 516..519 er-projection.
    def wext(W, al, ar, K):
        We = np.zeros((K, 520), f32)
        We[:, :512] = W
        for h in range(H):
            We[:, 512 + h] = W[:, h * D:(h + 1) * D] @ al[h]
            We[:, 516 + h] = W[:, h * D:(h + 1) * D] @ ar[h]
        # device layout [128, K//128, 520]
        return np.ascontiguousarray(
            We.reshape(K // 128, 128, 520).transpose(1, 0, 2)).astype(bf16)

    w0_host = wext(W0, al0, ar0, IN_DIM)
    w1_host = wext(W1, al1, ar1, H * D)
    ident = np.eye(128, dtype=bf16)

    # Per-core edge prep
    r_dst = dst // SH
    loc_dst = dst - r_dst * SH
    r_src = src // SH
    loc_src = src - r_src * SH
    # L1 global (chunk-major) row for every node
    c_src = loc_src // CHROWS
    srcg1_all = c_src * (NCORES * CHROWS) + r_src * CHROWS + (loc_src - c_src * CHROWS)

    cores = []
    counts_all = []
    for k in range(NCORES):
        sel = np.nonzero(r_dst == k)[0]
        dloc = loc_dst[sel]
        blk = dloc >> 7
        order = np.lexsort((src[sel], blk))
        sel = sel[order]
        dloc = dloc[order]
        blk = blk[order]
        counts = np.bincount(blk, minlength=NB)
        cores.append((sel, dloc, blk, counts))
        counts_all.append(counts)
    T_b = int(np.ceil(np.concatenate(counts_all).max() / 128))
    TT = NB * T_b
    T_slots = TT * 128

    in_maps = []
    for k in range(NCORES):
        sel, dloc, blk, counts = cores[k]
        csum = np.concatenate([[0], np.cumsum(counts)])
        rank_within = np.arange(len(blk)) - csum[blk]
        slot = blk * (T_b * 128) + rank_within

        srcg0 = np.full(T_slots, 6271, np.int32)          # pad -> own pad row (rotated layout)
        srcg1 = np.full(T_slots, 6 * NCORES * CHROWS + k * CHROWS + (RPC - 1 - 6 * CHROWS),
                        np.int32)                          # own pad row in L1 layout
        ohcol = np.full(T_slots, 127, np.int64)

        s = src[sel]
        rr = (r_src[sel] - k) % NCORES
        srcg0[slot] = rr * RPC + loc_src[sel]
        srcg1[slot] = srcg1_all[sel]
        ohcol[slot] = dloc - blk * 128

        ohf = np.zeros((T_slots, 128), f8)
        ohf[np.arange(T_slots), ohcol] = 1.0
        oh = ohf.reshape(TT, 128, 128)
        oht = np.ascontiguousarray(oh.transpose(0, 2, 1))

        # x transposed, rank-rotated so core k's shard is first: [128, 2, NPAD]
        xp = np.zeros((NPAD, IN_DIM), f32)
        for pos in range(NCORES):
            r = (k + pos) % NCORES
            xp[pos * RPC: pos * RPC + SH] = x[r * SH:(r + 1) * SH]
        xT = np.ascontiguousarray(
            xp.T.reshape(2, 128, NPAD).transpose(1, 0, 2)).astype(bf16)

        in_maps.append({
            "xT": xT,
            "w0": w0_host,
            "w1": w1_host,
            "ident": ident,
            "oh": oh,
            "oht": oht,
            "sg0": np.ascontiguousarray(srcg0.reshape(TT, 128).T),
            "sg1": np.ascontiguousarray(srcg1.reshape(TT, 128).T),
            "padel": np.full((22, 4), PAD_EL, bf16),
        })
    return in_maps, T_b


def _build(T_b):
    import concourse.bacc as bacc
    import concourse.bass as bass
    import concourse.mybir as mybir
    import concourse.tile as tile

    dt = mybir.dt
    TT = NB * T_b
    nc = bacc.Bacc("TRN2", target_bir_lowering=False, debug=False,
                   num_devices=NCORES)

    t_xT = nc.dram_tensor("xT", [128, 2, NPAD], dt.bfloat16, kind="ExternalInput")
    t_w0 = nc.dram_tensor("w0", [128, 2, 520], dt.bfloat16, kind="ExternalInput")
    t_w1 = nc.dram_tensor("w1", [128, 4, 520], dt.bfloat16, kind="ExternalInput")
    t_id = nc.dram_tensor("ident", [128, 128], dt.bfloat16, kind="ExternalInput")
    t_oh = nc.dram_tensor("oh", [TT, 128, 128], dt.float8e4, kind="ExternalInput")
    t_oht = nc.dram_tensor("oht", [TT, 128, 128], dt.float8e4, kind="ExternalInput")
    t_sg0 = nc.dram_tensor("sg0", [128, TT], dt.int32, kind="ExternalInput")
    t_sg1 = nc.dram_tensor("sg1", [128, TT], dt.int32, kind="ExternalInput")
    t_pad = nc.dram_tensor("padel", [22, 4], dt.bfloat16, kind="ExternalInput")
    t_out = nc.dram_tensor("out", [RPC, 128], dt.float32, kind="ExternalOutput")

    AF = mybir.ActivationFunctionType
    ALU = mybir.AluOpType

    with tile.TileContext(nc) as tc:
        with (
            tc.tile_pool(name="dram", bufs=1, space="DRAM") as dram,
            tc.tile_pool(name="res", bufs=1) as res,
            tc.tile_pool(name="xt", bufs=3) as xt_pool,
            tc.tile_pool(name="hx", bufs=4) as hx_pool,
            tc.tile_pool(name="hsrc", bufs=8) as hsrc_pool,
            tc.tile_pool(name="ohp", bufs=8) as oh_pool,
            tc.tile_pool(name="ohtp", bufs=8) as oht_pool,
            tc.tile_pool(name="msg", bufs=4) as msg_pool,
            tc.tile_pool(name="small", bufs=8) as small_pool,
            tc.tile_pool(name="x1t", bufs=9) as x1t_pool,
            tc.tile_pool(name="outp", bufs=4) as out_pool,
            tc.tile_pool(name="pbig", bufs=3, space="PSUM") as psum_big,
            tc.tile_pool(name="ptp", bufs=1, space="PSUM") as psum_tp,
            tc.tile_pool(name="psml", bufs=3, space="PSUM") as psum_small,
            tc.tile_pool(name="pden", bufs=1, space="PSUM") as psum_den,
        ):
            h_all0 = dram.tile([NPAD, COLS], dt.bfloat16)
            h1_own = dram.tile([RPC, COLS], dt.bfloat16)
            h_all1 = dram.tile([NPAD, COLS], dt.bfloat16)

            # resident tiles
            w0_sb = res.tile([128, 2, 520], dt.bfloat16)
            nc.sync.dma_start(w0_sb[:], t_w0.ap()[:])
            w1_sb = res.tile([128, 4, 520], dt.bfloat16)
            nc.sync.dma_start(w1_sb[:], t_w1.ap()[:])
            id_sb = res.tile([128, 128], dt.bfloat16)
            nc.sync.dma_start(id_sb[:], t_id.ap()[:])
            sg0_sb = res.tile([128, TT], dt.int32)
            nc.sync.dma_start(sg0_sb[:], t_sg0.ap()[:])
            sg1_sb = res.tile([128, TT], dt.int32)
            nc.sync.dma_start(sg1_sb[:], t_sg1.ap()[:])
            pad_sb = res.tile([22, 4], dt.bfloat16)
            nc.sync.dma_start(pad_sb[:], t_pad.ap()[:])
            er0_sb = res.tile([128, NB * 4], dt.bfloat16)
            er1_sb = res.tile([128, NB * 4], dt.bfloat16)

            def gemm_block(xt_ap, w_sb, kt, b_all, er_sb, b_own, h_dst):
                """One 128-node GEMM block: h cols + el/er cols."""
                ph = psum_big.tile([128, 512], dt.float32, name="ph", tag="big")
                pe = psum_small.tile([128, 8], dt.float32, name="pe", tag="sm")
                for kk in range(kt):
                    nc.tensor.matmul(ph[:], lhsT=xt_ap(kk), rhs=w_sb[:, kk, 0:512],
                                     start=(kk == 0), stop=(kk == kt - 1))
                for kk in range(kt):
                    nc.tensor.matmul(pe[:], lhsT=xt_ap(kk), rhs=w_sb[:, kk, 512:520],
                                     start=(kk == 0), stop=(kk == kt - 1))
                hx = hx_pool.tile([128, COLS], dt.bfloat16, name="hx")
                nc.scalar.copy(hx[:, 0:512], ph[:])
                nc.vector.tensor_copy(hx[:, 512:516], pe[:, 0:4])
                if b_own is not None:
                    nc.vector.tensor_copy(er_sb[:, b_own * 4:(b_own + 1) * 4],
                                          pe[:, 4:8])
                nc.sync.dma_start(h_dst, hx[:])

            # ---- GEMM0: replicated over all NPAD rows (rotated order) ----
            for sb in range(NPAD // 512):
                xt = xt_pool.tile([128, 2, 512], dt.bfloat16, name="xt")
                nc.sync.dma_start(xt[:], t_xT.ap()[:, :, sb * 512:(sb + 1) * 512])
                for cb in range(4):
                    b = sb * 4 + cb
                    gemm_block(
                        lambda kk: xt[:, kk, cb * 128:(cb + 1) * 128],
                        w0_sb, 2, b, er0_sb, b if b < NB else None,
                        h_all0[b * 128:(b + 1) * 128, :])

            for r in range(NCORES):
                nc.sync.dma_start(
                    h_all0[r * RPC + SH: r * RPC + RPC, 512:516], pad_sb[:])

            def edge_block(b, sg_sb, er_sb, h_tab):
                """Edge softmax + scatter for node block b. Returns (pm, rec)."""
                pm = psum_big.tile([128, 512], dt.float32, name="pm", tag="big")
                pd = psum_den.tile([128, 4], dt.float32, name="pd")
                for tt in range(T_b):
                    t = b * T_b + tt
                    hs = hsrc_pool.tile([128, COLS], dt.bfloat16, name="hs")
                    nc.gpsimd.indirect_dma_start(
                        out=hs[:], out_offset=None, in_=h_tab[:],
                        in_offset=bass.IndirectOffsetOnAxis(ap=sg_sb[:, t:t + 1],
                                                            axis=0))
                    oh_t = oh_pool.tile([128, 128], dt.float8e4, name="oh_t")
                    nc.sync.dma_start(oh_t[:], t_oh.ap()[t, :, :])
                    oht_t = oht_pool.tile([128, 128], dt.float8e4, name="oht_t")
                    nc.sync.dma_start(oht_t[:], t_oht.ap()[t, :, :])
                    # e_pre = er[dst] + el[src] via two matmuls into PSUM
                    pe2 = psum_small.tile([128, 4], dt.float32, name="pe2", tag="sm")
                    nc.tensor.matmul(pe2[:], lhsT=oht_t[:],
                                     rhs=er_sb[:, b * 4:(b + 1) * 4],
                                     start=True, stop=False)
                    nc.tensor.matmul(pe2[:], lhsT=id_sb[:], rhs=hs[:, 512:516],
                                     start=False, stop=True)
                    tmp = small_pool.tile([128, 4], dt.float32, name="tmp")
                    nc.scalar.activation(tmp[:], pe2[:], AF.Prelu, alpha=0.2)
                    exf = small_pool.tile([128, 4], dt.float32, name="exf")
                    nc.scalar.activation(exf[:], tmp[:], AF.Exp)
                    exb = small_pool.tile([128, 4], dt.bfloat16, name="exb")
                    nc.vector.tensor_copy(exb[:], exf[:])
                    ms = msg_pool.tile([128, 512], dt.bfloat16, name="ms")
                    for h in range(H):
                        nc.vector.tensor_scalar_mul(
                            ms[:, h * D:(h + 1) * D], hs[:, h * D:(h + 1) * D],
                            exf[:, h:h + 1])
                    nc.tensor.matmul(pm[:], lhsT=oh_t[:], rhs=ms[:],
                                     start=(tt == 0), stop=(tt == T_b - 1))
                    nc.tensor.matmul(pd[:], lhsT=oh_t[:], rhs=exb[:],
                                     start=(tt == 0), stop=(tt == T_b - 1))
                dcl = small_pool.tile([128, 4], dt.float32, name="dcl")
                nc.vector.tensor_scalar_max(dcl[:], pd[:], 1e-30)
                rec = small_pool.tile([128, 4], dt.float32, name="rec")
                nc.vector.reciprocal(rec[:], dcl[:])
                return pm, rec

            # ---- L0 edge phase, interleaved with GEMM1 + AllGather chunks ----
            x1t_tiles = {}
            for b in range(NB):
                pm, rec = edge_block(b, sg0_sb, er0_sb, h_all0)
                x1 = x1t_pool.tile([128, 512], dt.bfloat16, name="x1", tag="x1")
                for h in range(H):
                    nc.scalar.activation(x1[:, h * D:(h + 1) * D],
                                         pm[:, h * D:(h + 1) * D], AF.Relu,
                                         scale=rec[:, h:h + 1])
                tp = psum_tp.tile([128, 512], dt.bfloat16, name="tp")
                for h in range(H):
                    nc.tensor.transpose(out=tp[:, h * D:(h + 1) * D],
                                        in_=x1[:, h * D:(h + 1) * D],
                                        identity=id_sb[:])
                x1t = x1t_pool.tile([128, 512], dt.bfloat16, name="x1t", tag="x1t")
                nc.vector.tensor_copy(x1t[:], tp[:])
                x1t_tiles[b] = x1t
                if b % CHB == CHB - 1:
                    c = b // CHB
                    for bb in range(c * CHB, (c + 1) * CHB):
                        xt1 = x1t_tiles.pop(bb)
                        gemm_block(
                            lambda kk, _x=xt1: _x[:, kk * 128:(kk + 1) * 128],
                            w1_sb, 4, bb, er1_sb, bb,
                            h1_own[bb * 128:(bb + 1) * 128, :])
                    if c == CHB - 1:
                        nc.sync.dma_start(h1_own[SH:RPC, 512:516], pad_sb[:])
                    nc.gpsimd.collective_compute(
                        "AllGather", ALU.bypass,
                        replica_groups=[list(range(NCORES))],
                        ins=[h1_own[c * CHROWS:(c + 1) * CHROWS, :]],
                        outs=[h_all1[c * NCORES * CHROWS:(c + 1) * NCORES * CHROWS, :]],
                    )

            # ---- L1 edge phase -> mean over heads ----
            for b in range(NB):
                pm, rec = edge_block(b, sg1_sb, er1_sb, h_all1)
                rec4 = small_pool.tile([128, 4], dt.float32, name="rec4")
                nc.vector.tensor_scalar_mul(rec4[:], rec[:], 0.25)
                o = []
                for h in range(H):
                    oh_ = out_pool.tile([128, 128], dt.float32, name=f"o{h}",
                                        tag=f"o{h}")
                    nc.scalar.activation(oh_[:], pm[:, h * D:(h + 1) * D], AF.Relu,
                                         scale=rec4[:, h:h + 1])
                    o.append(oh_)
                s01 = out_pool.tile([128, 128], dt.float32, name="s01", tag="s01")
                nc.vector.tensor_tensor(out=s01[:], in0=o[0][:], in1=o[1][:],
                                        op=ALU.add)
                s23 = out_pool.tile([128, 128], dt.float32, name="s23", tag="s23")
                nc.vector.tensor_tensor(out=s23[:], in0=o[2][:], in1=o[3][:],
                                        op=ALU.add)
                so = out_pool.tile([128, 128], dt.float32, name="so", tag="so")
                nc.vector.tensor_tensor(out=so[:], in0=s01[:], in1=s23[:],
                                        op=ALU.add)
                nc.sync.dma_start(t_out.ap()[b * 128:(b + 1) * 128, :], so[:])

    nc.compile()
    return nc


def _get_nc(T_b):
    if T_b not in _cache:
        _cache[T_b] = _build(T_b)
    return _cache[T_b]


def kernel(x, src, dst, W0, al0, ar0, W1, al1, ar1):
    from concourse.bass_utils import run_bass_kernel_spmd
    in_maps, T_b = _host_prep(x, src, dst, W0, al0, ar0, W1, al1, ar1)
    nc = _get_nc(T_b)
    res = run_bass_kernel_spmd(nc, in_maps, core_ids=list(range(NCORES)))
    out = np.empty((N, D), np.float32)
    for k in range(NCORES):
        out[k * SH:(k + 1) * SH] = res.results[k]["out"][:SH]
    return out
